# revision 46
# baseline (speedup 1.0000x reference)
"""Trainium2 Bass kernel for causal multi-head attention (nn_Attention_5334349381821).

Problem: b=2, n=2048, dim=1024, 16 heads x 64 dim_head, RMSNorm + QKV + RoPE
(interleaved) + causal softmax attention + output projection.

Sharding: 8 cores = data-parallel on batch (2) x tensor-parallel on heads (4
groups of 4 heads). Each core computes a partial output [2048, 1024] through
its wo column-slice; host sums the 4 partials per batch element.

v2 design (bf16 + phase overlap), measured 257.0us/iter repeat-slope on HW
(prior fp32r baseline: 343.8us same-method), TimelineSim 172.1us:
  - all matmul operands bf16 (PSUM accumulation fp32); fp32 kept only on the
    RMSNorm scale path (ssq -> s) and PSUM tiles
  - norm_weight folded into wq/wk/wv on host
  - RMS scale s_i folded into q's RoPE tables (cos_s/sin_s); s_j folded into
    the softmax exp via per-partition activation scale AP (scale_t = s_j/sqrt(dh));
    v scaled by s_j via DVE tensor_scalar; squares for ssq on DVE (bf16 2x)
  - exp pair-fused: both heads' S tiles in one [128, 2x512] PSUM tile, one
    ACT exp instruction for full (non-diagonal) k-tiles; diagonal tiles split
    per-head (exp -> tri-mask -> AV without cross-head waits)
  - attention software-pipelined: AV(t-1) emitted after S(t)/exp(t) so PE
    streams the next S while ACT computes exp
  - RMS scale transposed to token-partition layout via 16 PE transposes
    (replaces a 2048x4B-descriptor DRAM gather bounce)
  - schedule: x/w loads -> sq/ssq -> v-proj -> minimal qk prefix (k chunks
    0,1 + q chunk 1, k-first) -> attention chunks in order 1,0,2,3; the
    remaining qk chains and each finished chunk's out-projection run as
    deferred PE filler chains drained inside later attentions' latency
    bubbles (diagonal strip + every 4th t); <= 8 PSUM banks throughout
  - gpsimd (Pool) never touches PSUM (HW restriction): PSUM reads/writes on
    PE/ACT/DVE only; out-proj staging copies on DVE (ACT for the tail chunk)
"""

from contextlib import ExitStack

import numpy as np

import concourse.bass as bass
import concourse.tile as tile
from concourse import bacc, mybir
from concourse.bass_utils import run_bass_kernel_spmd

# Problem constants (hardcoded; kernel.py must be self-contained)
B = 2
N = 2048
DIM = 1024
HEADS = 16
DH = 64
N_CORES = 8
HEADS_PER_CORE = HEADS // (N_CORES // B)  # 4
M = HEADS_PER_CORE * DH  # 256 = per-core q/k/v width
RMS_EPS = 1.1920929e-07
ROPE_THETA = 10000.0

P = 128
F32 = mybir.dt.float32
BF16 = mybir.dt.bfloat16

KT = DIM // P        # 8 k-tiles over dim
IT = N // P          # 16 token tiles of 128
NC = N // 512        # 4 chunks of 512 tokens
MT = M // P          # 2 m-tiles (= head-pairs)

SHUF_SWAP = [i ^ 1 for i in range(32)]
DEEP_POOLS = False  # deeper SBUF pools (model likes it; HW A/B showed regression)
REPEATS = 1  # emit the body multiple times (for repeat-slope HW timing)


def build_program():
    nc = bacc.Bacc(
        "TRN2",
        target_bir_lowering=False,
        debug=False,
        enable_asserts=False,
        num_devices=N_CORES,
    )

    xT_d = nc.dram_tensor("xT", [DIM, N], BF16, kind="ExternalInput").ap()
    wqT_d = nc.dram_tensor("wqT", [DIM, M], BF16, kind="ExternalInput").ap()
    wkT_d = nc.dram_tensor("wkT", [DIM, M], BF16, kind="ExternalInput").ap()
    wvT_d = nc.dram_tensor("wvT", [DIM, M], BF16, kind="ExternalInput").ap()
    woT_d = nc.dram_tensor("woT", [M, DIM], BF16, kind="ExternalInput").ap()
    cos_d = nc.dram_tensor("cos_t", [P, N], BF16, kind="ExternalInput").ap()
    sin_d = nc.dram_tensor("sin_t", [P, N], BF16, kind="ExternalInput").ap()
    tri_d = nc.dram_tensor("tri", [P, P], BF16, kind="ExternalInput").ap()
    out_d = nc.dram_tensor("out_part", [N, DIM], BF16, kind="ExternalOutput").ap()

    with tile.TileContext(nc) as tc:
        for _rep in range(REPEATS):
            _emit(nc, tc, xT_d, wqT_d, wkT_d, wvT_d, woT_d, cos_d, sin_d, tri_d, out_d)

    nc.compile()
    return nc


def _emit(nc, tc, xT_d, wqT_d, wkT_d, wvT_d, woT_d, cos_d, sin_d, tri_d, out_d):
    MULT = mybir.AluOpType.mult
    ADD = mybir.AluOpType.add
    EXPF = mybir.ActivationFunctionType.Exp
    COPYF = mybir.ActivationFunctionType.Copy
    SIGMA = DH ** -0.5

    with ExitStack() as whole:
        # ---------- long-lived pools ----------
        persist = whole.enter_context(tc.tile_pool(name="persist", bufs=1))

        tri = persist.tile([P, P], BF16, name="tri", tag="tri")
        ones_col = persist.tile([P, 1], BF16, name="ones_col", tag="ones_col")
        nc.vector.memset(ones_col[:], 1.0)
        # fp32 per-token-partition scale tiles: sT_col = s, scale_t = s/sqrt(dh)
        sT_col = persist.tile([P, IT], F32, name="sT_col", tag="sT_col")
        scale_t = persist.tile([P, IT], F32, name="scale_t", tag="scale_t")

        wo = persist.tile([P, MT, DIM], BF16, name="wo", tag="wo")

        qT = [persist.tile([P, N], BF16, name=f"qT{mt}", tag=f"qT{mt}") for mt in range(MT)]
        kTt = [persist.tile([P, N], BF16, name=f"kT{mt}", tag=f"kT{mt}") for mt in range(MT)]
        v_aug = persist.tile([P, IT, HEADS_PER_CORE, DH + 1], BF16, name="v_aug", tag="v_aug")
        OT = [persist.tile([P, N], BF16, name=f"OT{mt}", tag=f"OT{mt}") for mt in range(MT)]
        nc.vector.memset(v_aug[:, :, :, DH : DH + 1], 1.0)

        with ExitStack() as xphase:
            # ---------- loads (x first: gates ssq; weights right behind) ----------
            xpool = xphase.enter_context(tc.tile_pool(name="xpool", bufs=1))
            wv_sb = xpool.tile([P, KT, M], BF16, name="wv_sb", tag="wv_sb")
            wq_sb = xpool.tile([P, KT, M], BF16, name="wq_sb", tag="wq_sb")
            wk_sb = xpool.tile([P, KT, M], BF16, name="wk_sb", tag="wk_sb")
            xT = []
            for kt in range(KT):
                t = xpool.tile([P, N], BF16, name=f"xT{kt}", tag=f"xT{kt}")
                for half in range(2):
                    hs = slice(half * (N // 2), (half + 1) * (N // 2))
                    nc.sync.dma_start(t[:, hs], xT_d[kt * P : (kt + 1) * P, hs])
                xT.append(t)
            nc.sync.dma_start(wv_sb[:], wvT_d.rearrange("(o p) m -> p o m", p=P))
            nc.sync.dma_start(wq_sb[:], wqT_d.rearrange("(o p) m -> p o m", p=P))
            nc.sync.dma_start(wk_sb[:], wkT_d.rearrange("(o p) m -> p o m", p=P))
            cos_t = xpool.tile([P, N], BF16, name="cos_t", tag="cos")
            nc.sync.dma_start(cos_t[:], cos_d[:])
            sin_t = xpool.tile([P, N], BF16, name="sin_t", tag="sin")
            nc.sync.dma_start(sin_t[:], sin_d[:])
            nc.sync.dma_start(tri[:], tri_d[:])
            nc.sync.dma_start(wo[:], woT_d.rearrange("(o p) d -> p o d", p=P))
            # s-scaled rope tables for q
            cos_s = xpool.tile([P, N], BF16, name="cos_s", tag="cos_s")
            sin_s = xpool.tile([P, N], BF16, name="sin_s", tag="sin_s")

            # ---------- phase 1: RMSNorm scale ----------
            with tc.tile_pool(name="ph1", bufs=1) as ph1, \
                 tc.tile_pool(name="sqpool", bufs=(4 if DEEP_POOLS else 3)) as sqpool, \
                 tc.tile_pool(name="ps_ssq", bufs=1, space="PSUM") as ps_ssq:
                s_row = ph1.tile([1, N], F32, name="s_row", tag="s_row")
                eps_t = ph1.tile([1, 1], F32, name="eps_t", tag="eps_t")
                nc.vector.memset(eps_t[:], RMS_EPS)
                s_bcast = ph1.tile([P, N], F32, name="s_bcast", tag="s_bcast")
                ssq_ps = [
                    ps_ssq.tile([1, 512], F32, name=f"ssq{c}", tag=f"ssq{c}")
                    for c in range(NC)
                ]
                for kt in range(KT):
                    sq = sqpool.tile([P, N], BF16, name="sq", tag="sq")
                    for half in range(2):
                        hh = slice(half * (N // 2), (half + 1) * (N // 2))
                        nc.vector.tensor_tensor(sq[:, hh], xT[kt][:, hh], xT[kt][:, hh], MULT)
                    for c in range(NC):
                        cs = slice(c * 512, (c + 1) * 512)
                        nc.tensor.matmul(
                            ssq_ps[c][:], (ones_col), (sq[:, cs]),
                            start=(kt == 0), stop=(kt == KT - 1),
                        )
                for c in range(NC):
                    cs = slice(c * 512, (c + 1) * 512)
                    rt = ph1.tile([1, 512], F32, name="rt", tag="rt")
                    nc.scalar.activation(
                        rt[:], ssq_ps[c][:], mybir.ActivationFunctionType.Sqrt,
                        bias=eps_t[:], scale=1.0 / DIM,
                    )
                    nc.vector.reciprocal(s_row[:, cs], rt[:])

                # s into token-partition layout via 16 PE transposes of
                # [1,128] chunks (avoids the 2048x4B-descriptor DRAM gather,
                # which the cost model prices optimistically at 7ns/desc)
                ones_f32 = ph1.tile([1, 1], F32, name="ones_f32", tag="ones_f32")
                nc.vector.memset(ones_f32[:], 1.0)
                with tc.tile_pool(name="ps_st", bufs=1, space="PSUM") as ps_st:
                    sT_ps = ps_st.tile([P, IT], F32, name="sT_ps", tag="sT_ps")
                    for t in range(IT):
                        nc.tensor.transpose(
                            sT_ps[:, t : t + 1],
                            s_row[0:1, t * P : (t + 1) * P],
                            ones_f32[:],
                        )
                    nc.scalar.activation(sT_col[:], sT_ps[:], COPYF)
                nc.scalar.mul(scale_t[:], sT_col[:], SIGMA)

                nc.gpsimd.partition_broadcast(s_bcast[:], s_row[:])
                nc.vector.tensor_tensor(cos_s[:], cos_t[:], s_bcast[:], MULT)
                nc.vector.tensor_tensor(sin_s[:], sin_t[:], s_bcast[:], MULT)

            # ---------- phase 2: v projection ----------
            with tc.tile_pool(name="ps_v", bufs=6, space="PSUM") as ps_v:
                for jt in range(IT):
                    vp = ps_v.tile([P, M], F32, name=f"v_ps{jt}", tag="v_ps")
                    for kt in range(KT):
                        nc.tensor.matmul(
                            vp[:],
                            (xT[kt][:, jt * P : (jt + 1) * P]),
                            (wv_sb[:, kt, :]),
                            start=(kt == 0), stop=(kt == KT - 1),
                        )
                    # v scaled by s_j (per-partition scalar)
                    nc.vector.tensor_scalar_mul(
                        v_aug[:, jt, :, 0:DH],
                        vp.rearrange("p (h e) -> p h e", h=HEADS_PER_CORE),
                        sT_col[:, jt : jt + 1],
                    )

            # ---------- phase 3+4 interleaved: qk-proj(c) then attention(c) ----------
            # PSUM budget (16KB/partition): ps_gen (qk-proj + out-proj accums,
            # phase-disjoint, same shape/tag) 2x2KB + ps_s 2x4KB + ps_o 2x2KB
            with tc.tile_pool(name="ps_gen", bufs=2, space="PSUM") as ps_gen, \
                 tc.tile_pool(name="rope", bufs=(6 if DEEP_POOLS else 3)) as rope, \
                 tc.tile_pool(name="ppool", bufs=(6 if DEEP_POOLS else 3)) as ppool, \
                 tc.tile_pool(name="dpool", bufs=(4 if DEEP_POOLS else 2)) as dpool, \
                 tc.tile_pool(name="ps_s", bufs=2, space="PSUM") as ps_s, \
                 tc.tile_pool(name="ps_o", bufs=1, space="PSUM") as ps_o, \
                 tc.tile_pool(name="opool", bufs=(8 if DEEP_POOLS else 4)) as opool:

                def qk_chain(c, w, dst, ct, st, mt):
                    # one q-or-k projection 8-matmul chain + RoPE, as a
                    # deferred closure usable as PE filler inside attention
                    def go():
                        cs = slice(c * 512, (c + 1) * 512)
                        acc = ps_gen.tile([P, 512], F32, name=f"prj{mt}_{c}", tag="gen")
                        for kt in range(KT):
                            nc.tensor.matmul(
                                acc[:], (w[:, kt, mt * P : (mt + 1) * P]),
                                (xT[kt][:, cs]),
                                start=(kt == 0), stop=(kt == KT - 1),
                            )
                        raw = rope.tile([P, 512], BF16, name="raw", tag="raw")
                        nc.scalar.activation(raw[:], acc[:], COPYF)
                        sw = rope.tile([P, 512], BF16, name="sw", tag="sw")
                        nc.vector.stream_shuffle(sw[:], raw[:], SHUF_SWAP)
                        t1 = rope.tile([P, 512], BF16, name="t1", tag="t1")
                        nc.vector.tensor_tensor(t1[:], raw[:], ct[:, cs], MULT)
                        nc.vector.tensor_tensor(sw[:], sw[:], st[:, cs], MULT)
                        nc.gpsimd.tensor_tensor(dst[mt][:, cs], t1[:], sw[:], ADD)
                    return go

                def qk_chains(c):
                    return [
                        qk_chain(c, w, dst, ct, st, mt)
                        for w, dst, ct, st in (
                            (wq_sb, qT, cos_s, sin_s),
                            (wk_sb, kTt, cos_t, sin_t),
                        )
                        for mt in range(MT)
                    ]

                filler_q = []

                def drain_fillers(k):
                    for _ in range(min(k, len(filler_q))):
                        filler_q.pop(0)()

                def attention(c, hp):
                    pO = [
                        ps_o.tile([DH + 1, 512], F32, name=f"o{h}_{hp}_{c}", tag=f"pO{h}")
                        for h in range(2)
                    ]
                    tmax = 4 * (c + 1)

                    def emit_av(t, pt):
                        off = max(0, t - 4 * c) * P
                        nt = 512 - off
                        for h in range(2):
                            nc.tensor.matmul(
                                pO[h][:, off : off + nt],
                                (v_aug[:, t, hp * 2 + h, :]),
                                (pt[:, h, 0:nt]),
                                start=(t == 0), stop=(t == tmax - 1),
                                skip_group_check=True,
                            )

                    # software pipeline: AV(t-1) is emitted after S(t)/exp(t)
                    # so PE streams S(t+1) while ACT computes exp(t)
                    pending = None
                    for t in range(tmax):
                        # drain PE fillers where attention is latency-bound:
                        # the diagonal strip, plus sparsely in the full region
                        if t >= 4 * c or t % 4 == 1:
                            drain_fillers(1)
                        off = max(0, t - 4 * c) * P
                        nt = 512 - off
                        i_lo = c * 512 + off
                        pS = ps_s.tile([P, 2, 512], F32, name="spair", tag="spair")
                        for h in range(2):
                            base = h * DH
                            nc.tensor.matmul(
                                pS[:, h, 0:nt],
                                (kTt[hp][base : base + DH, t * P : (t + 1) * P]),
                                (qT[hp][base : base + DH, i_lo : (c + 1) * 512]),
                                start=True, stop=True,
                                tile_position=(base, 0),
                            )
                        pt = ppool.tile([P, 2, 512], BF16, name="pt", tag="pt")
                        if t < 4 * c:
                            # full tile: one fused exp over both heads
                            nc.scalar.activation(
                                pt[:, :, :], pS[:, :, :], EXPF,
                                scale=scale_t[:, t : t + 1],
                            )
                        else:
                            # diagonal tile: per-head exp + mask
                            for h in range(2):
                                nc.scalar.activation(
                                    pt[:, h, 0:nt], pS[:, h, 0:nt], EXPF,
                                    scale=scale_t[:, t : t + 1],
                                )
                                nc.vector.tensor_tensor(
                                    pt[:, h, 0:P], pt[:, h, 0:P],
                                    tri[:, :], MULT,
                                )
                        if pending is not None:
                            emit_av(*pending)
                        pending = (t, pt)
                    emit_av(*pending)
                    for h in range(2):
                        den = dpool.tile([1, 512], F32, name="den", tag="den")
                        nc.vector.reciprocal(den[:], pO[h][DH : DH + 1, :])
                        recb = dpool.tile([DH, 512], F32, name="recb", tag="recb")
                        nc.gpsimd.partition_broadcast(recb[:], den[:])
                        nc.vector.tensor_tensor(
                            OT[hp][h * DH : (h + 1) * DH, c * 512 : (c + 1) * 512],
                            pO[h][0:DH, :], recb[:], MULT,
                        )

                def outproj_chain(it, dc, stage_eng):
                    def go():
                        po = ps_gen.tile([P, 512], F32, name=f"out{it}_{dc}", tag="gen")
                        for et in range(MT):
                            nc.tensor.matmul(
                                po[:],
                                (OT[et][:, it * P : (it + 1) * P]),
                                (wo[:, et, dc * 512 : (dc + 1) * 512]),
                                start=(et == 0), stop=(et == MT - 1),
                            )
                        osb = opool.tile([P, 512], BF16, name="osb", tag="osb")
                        if stage_eng == "dve":
                            nc.vector.tensor_copy(osb[:], po[:])
                        elif stage_eng == "act":
                            nc.scalar.activation(osb[:], po[:], COPYF)
                        else:
                            nc.gpsimd.tensor_copy(osb[:], po[:])
                        nc.sync.dma_start(
                            out_d[it * P : (it + 1) * P, dc * 512 : (dc + 1) * 512],
                            osb[:],
                        )
                    return go

                def outproj_chains(cp, engs=("dve",)):  # gpsimd cannot read PSUM
                    return [
                        outproj_chain(it, dc, engs[(it + dc) % len(engs)])
                        for it in range(cp * 4, (cp + 1) * 4)
                        for dc in range(DIM // 512)
                    ]

                # chunk order 1,0,2,3: attention(1,hp) starts once only ITS
                # k chunks 0,1 + q chunk 1 are projected (6 chains, k-first);
                # the q(0) chains, qk(2,3) and each finished chunk's out-proj
                # drain as PE fillers inside later attentions' latency bubbles
                def qch(c, mt):
                    return qk_chain(c, wq_sb, qT, cos_s, sin_s, mt)

                def kch(c, mt):
                    return qk_chain(c, wk_sb, kTt, cos_t, sin_t, mt)

                for chain in (kch(0, 0), kch(1, 0), qch(1, 0),
                              kch(0, 1), kch(1, 1), qch(1, 1)):
                    chain()
                filler_q.extend([qch(0, 0), qch(0, 1)])
                filler_q.extend(qk_chains(2) + qk_chains(3))
                for hp in range(MT):
                    attention(1, hp)
                drain_fillers(len(filler_q))  # qk(2,3) complete
                filler_q.extend(outproj_chains(1))
                for hp in range(MT):
                    attention(0, hp)
                filler_q.extend(outproj_chains(0))
                for hp in range(MT):
                    attention(2, hp)
                filler_q.extend(outproj_chains(2))
                for hp in range(MT):
                    attention(3, hp)
                drain_fillers(len(filler_q))
                for chain in outproj_chains(3, ("act", "dve")):
                    chain()


def _rope_tables():
    inv_freq = 1.0 / (ROPE_THETA ** (np.arange(0, DH, 2, dtype=np.float64) / DH))
    t = np.arange(N, dtype=np.float64)
    freqs = t[:, None] * inv_freq[None, :]  # [N, 32]
    cos = np.cos(freqs)
    sin = np.sin(freqs)
    rows = np.arange(P)
    tidx = (rows % DH) // 2
    cos_t = cos[:, tidx].T.astype(np.float32)  # [128, N]
    sign = np.where(rows % 2 == 0, -1.0, 1.0)
    sin_t = (sin[:, tidx] * sign[None, :]).T.astype(np.float32)
    return np.ascontiguousarray(cos_t), np.ascontiguousarray(sin_t)


def shard_inputs(tokens, norm_weight, wq, wk, wv, wo):
    """Build the 8 per-core input dicts (pure numpy layout prep)."""
    import ml_dtypes

    bf16 = ml_dtypes.bfloat16
    tokens = np.asarray(tokens, dtype=np.float32)
    norm_weight = np.asarray(norm_weight, dtype=np.float32)
    wq, wk, wv, wo = (np.asarray(w, dtype=np.float32) for w in (wq, wk, wv, wo))
    # fold norm_weight into the input side of the qkv projections
    wq_f = wq * norm_weight[None, :]
    wk_f = wk * norm_weight[None, :]
    wv_f = wv * norm_weight[None, :]

    cos_t, sin_t = _rope_tables()
    tri = np.triu(np.ones((P, P), dtype=np.float32))  # keep j <= i (row=j, col=i)

    in_maps = []
    for c in range(N_CORES):
        b = c // (N_CORES // B)
        g = c % (N_CORES // B)
        sl = slice(g * M, (g + 1) * M)
        in_maps.append({
            "xT": np.ascontiguousarray(tokens[b].T).astype(bf16),
            "wqT": np.ascontiguousarray(wq_f[sl, :].T).astype(bf16),
            "wkT": np.ascontiguousarray(wk_f[sl, :].T).astype(bf16),
            "wvT": np.ascontiguousarray(wv_f[sl, :].T).astype(bf16),
            "woT": np.ascontiguousarray(wo[:, sl].T).astype(bf16),
            "cos_t": cos_t.astype(bf16),
            "sin_t": sin_t.astype(bf16),
            "tri": tri.astype(bf16),
        })
    return in_maps


_PROGRAM = None


def _get_program():
    global _PROGRAM
    if _PROGRAM is None:
        _PROGRAM = build_program()
    return _PROGRAM


def run(tokens, norm_weight, wq, wk, wv, wo, trace=False, **run_kwargs):
    nc = _get_program()
    in_maps = shard_inputs(tokens, norm_weight, wq, wk, wv, wo)
    res = run_bass_kernel_spmd(
        nc, in_maps, core_ids=list(range(N_CORES)), trace=trace, **run_kwargs
    )
    parts = [r["out_part"] for r in res.results]
    out = np.zeros((B, N, DIM), dtype=np.float64)
    for c in range(N_CORES):
        out[c // (N_CORES // B)] += parts[c].astype(np.float64)
    return out.astype(np.float32), res


def kernel(tokens, norm_weight, wq, wk, wv, wo):
    out, _ = run(tokens, norm_weight, wq, wk, wv, wo)
    return out


if __name__ == "__main__":
    from concourse.timeline_sim import TimelineSim

    nc = build_program()
    ts = TimelineSim(nc, trace=False)
    print(f"TimelineSim: {ts.simulate():.0f} ns")


# revision 51
# speedup vs baseline: 1.0202x; 1.0202x over previous
"""Trainium2 Bass kernel for causal multi-head attention (nn_Attention_5334349381821).

Problem: b=2, n=2048, dim=1024, 16 heads x 64 dim_head, RMSNorm + QKV + RoPE
(interleaved) + causal softmax attention + output projection.

Sharding: 8 cores = data-parallel on batch (2) x tensor-parallel on heads (4
groups of 4 heads). Each core computes a partial output [2048, 1024] through
its wo column-slice; host sums the 4 partials per batch element.

v2 design (bf16 + phase overlap), measured 251.9us/iter repeat-slope on HW
(prior fp32r baseline: 343.8us same-method), TimelineSim 171.8us:
  - all matmul operands bf16 (PSUM accumulation fp32); fp32 kept only on the
    RMSNorm scale path (ssq -> s) and PSUM tiles
  - norm_weight folded into wq/wk/wv on host
  - RMS scale s_i folded into q's RoPE tables (cos_s/sin_s); s_j folded into
    the softmax exp via per-partition activation scale AP (scale_t = s_j/sqrt(dh));
    v scaled by s_j via DVE tensor_scalar; squares for ssq on DVE (bf16 2x)
  - exp pair-fused: both heads' S tiles in one [128, 2x512] PSUM tile, one
    ACT exp instruction for full (non-diagonal) k-tiles; diagonal tiles split
    per-head (exp -> tri-mask -> AV without cross-head waits)
  - attention software-pipelined: AV(t-1) emitted after S(t)/exp(t) so PE
    streams the next S while ACT computes exp
  - RMS scale transposed to token-partition layout via 16 PE transposes
    (replaces a 2048x4B-descriptor DRAM gather bounce)
  - schedule: x/w loads -> sq/ssq -> v-proj -> minimal qk prefix (k chunks
    0,1 + q chunk 1, k-first) -> attention chunks in order 1,0,2,3; the
    remaining qk chains and each finished chunk's out-projection run as
    deferred PE filler chains drained inside later attentions' latency
    bubbles (diagonal strip + every 4th t); <= 8 PSUM banks throughout
  - gpsimd (Pool) never touches PSUM (HW restriction): PSUM reads/writes on
    PE/ACT/DVE only; out-proj staging copies on DVE (ACT for the tail chunk);
    RoPE add on DVE (bf16 2x beats Pool's 0.42-efficiency path)
"""

from contextlib import ExitStack

import numpy as np

import concourse.bass as bass
import concourse.tile as tile
from concourse import bacc, mybir
from concourse.bass_utils import run_bass_kernel_spmd

# Problem constants (hardcoded; kernel.py must be self-contained)
B = 2
N = 2048
DIM = 1024
HEADS = 16
DH = 64
N_CORES = 8
HEADS_PER_CORE = HEADS // (N_CORES // B)  # 4
M = HEADS_PER_CORE * DH  # 256 = per-core q/k/v width
RMS_EPS = 1.1920929e-07
ROPE_THETA = 10000.0

P = 128
F32 = mybir.dt.float32
BF16 = mybir.dt.bfloat16

KT = DIM // P        # 8 k-tiles over dim
IT = N // P          # 16 token tiles of 128
NC = N // 512        # 4 chunks of 512 tokens
MT = M // P          # 2 m-tiles (= head-pairs)

SHUF_SWAP = [i ^ 1 for i in range(32)]
DEEP_POOLS = False  # deeper SBUF pools (model likes it; HW A/B showed regression)
REPEATS = 1  # emit the body multiple times (for repeat-slope HW timing)


def build_program():
    nc = bacc.Bacc(
        "TRN2",
        target_bir_lowering=False,
        debug=False,
        enable_asserts=False,
        num_devices=N_CORES,
    )

    xT_d = nc.dram_tensor("xT", [DIM, N], BF16, kind="ExternalInput").ap()
    wqT_d = nc.dram_tensor("wqT", [DIM, M], BF16, kind="ExternalInput").ap()
    wkT_d = nc.dram_tensor("wkT", [DIM, M], BF16, kind="ExternalInput").ap()
    wvT_d = nc.dram_tensor("wvT", [DIM, M], BF16, kind="ExternalInput").ap()
    woT_d = nc.dram_tensor("woT", [M, DIM], BF16, kind="ExternalInput").ap()
    cos_d = nc.dram_tensor("cos_t", [P, N], BF16, kind="ExternalInput").ap()
    sin_d = nc.dram_tensor("sin_t", [P, N], BF16, kind="ExternalInput").ap()
    tri_d = nc.dram_tensor("tri", [P, P], BF16, kind="ExternalInput").ap()
    out_d = nc.dram_tensor("out_part", [N, DIM], BF16, kind="ExternalOutput").ap()

    with tile.TileContext(nc) as tc:
        for _rep in range(REPEATS):
            _emit(nc, tc, xT_d, wqT_d, wkT_d, wvT_d, woT_d, cos_d, sin_d, tri_d, out_d)

    nc.compile()
    return nc


def _emit(nc, tc, xT_d, wqT_d, wkT_d, wvT_d, woT_d, cos_d, sin_d, tri_d, out_d):
    MULT = mybir.AluOpType.mult
    ADD = mybir.AluOpType.add
    EXPF = mybir.ActivationFunctionType.Exp
    COPYF = mybir.ActivationFunctionType.Copy
    SIGMA = DH ** -0.5

    with ExitStack() as whole:
        # ---------- long-lived pools ----------
        persist = whole.enter_context(tc.tile_pool(name="persist", bufs=1))

        tri = persist.tile([P, P], BF16, name="tri", tag="tri")
        ones_col = persist.tile([P, 1], BF16, name="ones_col", tag="ones_col")
        nc.vector.memset(ones_col[:], 1.0)
        # fp32 per-token-partition scale tiles: sT_col = s, scale_t = s/sqrt(dh)
        sT_col = persist.tile([P, IT], F32, name="sT_col", tag="sT_col")
        scale_t = persist.tile([P, IT], F32, name="scale_t", tag="scale_t")

        wo = persist.tile([P, MT, DIM], BF16, name="wo", tag="wo")

        qT = [persist.tile([P, N], BF16, name=f"qT{mt}", tag=f"qT{mt}") for mt in range(MT)]
        kTt = [persist.tile([P, N], BF16, name=f"kT{mt}", tag=f"kT{mt}") for mt in range(MT)]
        v_aug = persist.tile([P, IT, HEADS_PER_CORE, DH + 1], BF16, name="v_aug", tag="v_aug")
        OT = [persist.tile([P, N], BF16, name=f"OT{mt}", tag=f"OT{mt}") for mt in range(MT)]
        nc.vector.memset(v_aug[:, :, :, DH : DH + 1], 1.0)

        with ExitStack() as xphase:
            # ---------- loads (x first: gates ssq; weights right behind) ----------
            xpool = xphase.enter_context(tc.tile_pool(name="xpool", bufs=1))
            wv_sb = xpool.tile([P, KT, M], BF16, name="wv_sb", tag="wv_sb")
            wq_sb = xpool.tile([P, KT, M], BF16, name="wq_sb", tag="wq_sb")
            wk_sb = xpool.tile([P, KT, M], BF16, name="wk_sb", tag="wk_sb")
            xT = []
            for kt in range(KT):
                t = xpool.tile([P, N], BF16, name=f"xT{kt}", tag=f"xT{kt}")
                for half in range(2):
                    hs = slice(half * (N // 2), (half + 1) * (N // 2))
                    nc.sync.dma_start(t[:, hs], xT_d[kt * P : (kt + 1) * P, hs])
                xT.append(t)
            nc.sync.dma_start(wv_sb[:], wvT_d.rearrange("(o p) m -> p o m", p=P))
            nc.sync.dma_start(wq_sb[:], wqT_d.rearrange("(o p) m -> p o m", p=P))
            nc.sync.dma_start(wk_sb[:], wkT_d.rearrange("(o p) m -> p o m", p=P))
            cos_t = xpool.tile([P, N], BF16, name="cos_t", tag="cos")
            nc.sync.dma_start(cos_t[:], cos_d[:])
            sin_t = xpool.tile([P, N], BF16, name="sin_t", tag="sin")
            nc.sync.dma_start(sin_t[:], sin_d[:])
            nc.sync.dma_start(tri[:], tri_d[:])
            nc.sync.dma_start(wo[:], woT_d.rearrange("(o p) d -> p o d", p=P))
            # s-scaled rope tables for q
            cos_s = xpool.tile([P, N], BF16, name="cos_s", tag="cos_s")
            sin_s = xpool.tile([P, N], BF16, name="sin_s", tag="sin_s")

            # ---------- phase 1: RMSNorm scale ----------
            with tc.tile_pool(name="ph1", bufs=1) as ph1, \
                 tc.tile_pool(name="sqpool", bufs=(4 if DEEP_POOLS else 3)) as sqpool, \
                 tc.tile_pool(name="ps_ssq", bufs=1, space="PSUM") as ps_ssq:
                s_row = ph1.tile([1, N], F32, name="s_row", tag="s_row")
                eps_t = ph1.tile([1, 1], F32, name="eps_t", tag="eps_t")
                nc.vector.memset(eps_t[:], RMS_EPS)
                s_bcast = ph1.tile([P, N], F32, name="s_bcast", tag="s_bcast")
                ssq_ps = [
                    ps_ssq.tile([1, 512], F32, name=f"ssq{c}", tag=f"ssq{c}")
                    for c in range(NC)
                ]
                for kt in range(KT):
                    sq = sqpool.tile([P, N], BF16, name="sq", tag="sq")
                    for half in range(2):
                        hh = slice(half * (N // 2), (half + 1) * (N // 2))
                        nc.vector.tensor_tensor(sq[:, hh], xT[kt][:, hh], xT[kt][:, hh], MULT)
                    for c in range(NC):
                        cs = slice(c * 512, (c + 1) * 512)
                        nc.tensor.matmul(
                            ssq_ps[c][:], (ones_col), (sq[:, cs]),
                            start=(kt == 0), stop=(kt == KT - 1),
                        )
                for c in range(NC):
                    cs = slice(c * 512, (c + 1) * 512)
                    rt = ph1.tile([1, 512], F32, name="rt", tag="rt")
                    nc.scalar.activation(
                        rt[:], ssq_ps[c][:], mybir.ActivationFunctionType.Sqrt,
                        bias=eps_t[:], scale=1.0 / DIM,
                    )
                    nc.vector.reciprocal(s_row[:, cs], rt[:])

                # s into token-partition layout via 16 PE transposes of
                # [1,128] chunks (avoids the 2048x4B-descriptor DRAM gather,
                # which the cost model prices optimistically at 7ns/desc)
                ones_f32 = ph1.tile([1, 1], F32, name="ones_f32", tag="ones_f32")
                nc.vector.memset(ones_f32[:], 1.0)
                with tc.tile_pool(name="ps_st", bufs=1, space="PSUM") as ps_st:
                    sT_ps = ps_st.tile([P, IT], F32, name="sT_ps", tag="sT_ps")
                    for t in range(IT):
                        nc.tensor.transpose(
                            sT_ps[:, t : t + 1],
                            s_row[0:1, t * P : (t + 1) * P],
                            ones_f32[:],
                        )
                    nc.scalar.activation(sT_col[:], sT_ps[:], COPYF)
                nc.scalar.mul(scale_t[:], sT_col[:], SIGMA)

                nc.gpsimd.partition_broadcast(s_bcast[:], s_row[:])
                nc.vector.tensor_tensor(cos_s[:], cos_t[:], s_bcast[:], MULT)
                nc.vector.tensor_tensor(sin_s[:], sin_t[:], s_bcast[:], MULT)

            # ---------- phase 2: v projection ----------
            with tc.tile_pool(name="ps_v", bufs=6, space="PSUM") as ps_v:
                for jt in range(IT):
                    vp = ps_v.tile([P, M], F32, name=f"v_ps{jt}", tag="v_ps")
                    for kt in range(KT):
                        nc.tensor.matmul(
                            vp[:],
                            (xT[kt][:, jt * P : (jt + 1) * P]),
                            (wv_sb[:, kt, :]),
                            start=(kt == 0), stop=(kt == KT - 1),
                        )
                    # v scaled by s_j (per-partition scalar)
                    nc.vector.tensor_scalar_mul(
                        v_aug[:, jt, :, 0:DH],
                        vp.rearrange("p (h e) -> p h e", h=HEADS_PER_CORE),
                        sT_col[:, jt : jt + 1],
                    )

            # ---------- phase 3+4 interleaved: qk-proj(c) then attention(c) ----------
            # PSUM budget (16KB/partition): ps_gen (qk-proj + out-proj accums,
            # phase-disjoint, same shape/tag) 2x2KB + ps_s 2x4KB + ps_o 2x2KB
            with tc.tile_pool(name="ps_gen", bufs=2, space="PSUM") as ps_gen, \
                 tc.tile_pool(name="rope", bufs=(6 if DEEP_POOLS else 3)) as rope, \
                 tc.tile_pool(name="ppool", bufs=(6 if DEEP_POOLS else 3)) as ppool, \
                 tc.tile_pool(name="dpool", bufs=(4 if DEEP_POOLS else 2)) as dpool, \
                 tc.tile_pool(name="ps_s", bufs=2, space="PSUM") as ps_s, \
                 tc.tile_pool(name="ps_o", bufs=1, space="PSUM") as ps_o, \
                 tc.tile_pool(name="opool", bufs=(8 if DEEP_POOLS else 4)) as opool:

                def qk_chain(c, w, dst, ct, st, mt):
                    # one q-or-k projection 8-matmul chain + RoPE, as a
                    # deferred closure usable as PE filler inside attention
                    def go():
                        cs = slice(c * 512, (c + 1) * 512)
                        acc = ps_gen.tile([P, 512], F32, name=f"prj{mt}_{c}", tag="gen")
                        for kt in range(KT):
                            nc.tensor.matmul(
                                acc[:], (w[:, kt, mt * P : (mt + 1) * P]),
                                (xT[kt][:, cs]),
                                start=(kt == 0), stop=(kt == KT - 1),
                            )
                        raw = rope.tile([P, 512], BF16, name="raw", tag="raw")
                        nc.scalar.activation(raw[:], acc[:], COPYF)
                        sw = rope.tile([P, 512], BF16, name="sw", tag="sw")
                        nc.vector.stream_shuffle(sw[:], raw[:], SHUF_SWAP)
                        t1 = rope.tile([P, 512], BF16, name="t1", tag="t1")
                        nc.vector.tensor_tensor(t1[:], raw[:], ct[:, cs], MULT)
                        nc.vector.tensor_tensor(sw[:], sw[:], st[:, cs], MULT)
                        nc.vector.tensor_tensor(dst[mt][:, cs], t1[:], sw[:], ADD)
                    return go

                def qk_chains(c):
                    return [
                        qk_chain(c, w, dst, ct, st, mt)
                        for w, dst, ct, st in (
                            (wq_sb, qT, cos_s, sin_s),
                            (wk_sb, kTt, cos_t, sin_t),
                        )
                        for mt in range(MT)
                    ]

                filler_q = []

                def drain_fillers(k):
                    for _ in range(min(k, len(filler_q))):
                        filler_q.pop(0)()

                def attention(c, hp):
                    pO = [
                        ps_o.tile([DH + 1, 512], F32, name=f"o{h}_{hp}_{c}", tag=f"pO{h}")
                        for h in range(2)
                    ]
                    tmax = 4 * (c + 1)

                    def emit_av(t, pt):
                        off = max(0, t - 4 * c) * P
                        nt = 512 - off
                        for h in range(2):
                            nc.tensor.matmul(
                                pO[h][:, off : off + nt],
                                (v_aug[:, t, hp * 2 + h, :]),
                                (pt[:, h, 0:nt]),
                                start=(t == 0), stop=(t == tmax - 1),
                                skip_group_check=True,
                            )

                    # software pipeline: AV(t-1) is emitted after S(t)/exp(t)
                    # so PE streams S(t+1) while ACT computes exp(t)
                    pending = None
                    for t in range(tmax):
                        # drain PE fillers where attention is latency-bound:
                        # the diagonal strip, plus sparsely in the full region
                        if t >= 4 * c or t % 4 == 1:
                            drain_fillers(1)
                        off = max(0, t - 4 * c) * P
                        nt = 512 - off
                        i_lo = c * 512 + off
                        pS = ps_s.tile([P, 2, 512], F32, name="spair", tag="spair")
                        for h in range(2):
                            base = h * DH
                            nc.tensor.matmul(
                                pS[:, h, 0:nt],
                                (kTt[hp][base : base + DH, t * P : (t + 1) * P]),
                                (qT[hp][base : base + DH, i_lo : (c + 1) * 512]),
                                start=True, stop=True,
                                tile_position=(base, 0),
                            )
                        pt = ppool.tile([P, 2, 512], BF16, name="pt", tag="pt")
                        if t < 4 * c:
                            # full tile: one fused exp over both heads
                            nc.scalar.activation(
                                pt[:, :, :], pS[:, :, :], EXPF,
                                scale=scale_t[:, t : t + 1],
                            )
                        else:
                            # diagonal tile: per-head exp + mask
                            for h in range(2):
                                nc.scalar.activation(
                                    pt[:, h, 0:nt], pS[:, h, 0:nt], EXPF,
                                    scale=scale_t[:, t : t + 1],
                                )
                                nc.vector.tensor_tensor(
                                    pt[:, h, 0:P], pt[:, h, 0:P],
                                    tri[:, :], MULT,
                                )
                        if pending is not None:
                            emit_av(*pending)
                        pending = (t, pt)
                    emit_av(*pending)
                    for h in range(2):
                        den = dpool.tile([1, 512], F32, name="den", tag="den")
                        nc.vector.reciprocal(den[:], pO[h][DH : DH + 1, :])
                        recb = dpool.tile([DH, 512], F32, name="recb", tag="recb")
                        nc.gpsimd.partition_broadcast(recb[:], den[:])
                        nc.vector.tensor_tensor(
                            OT[hp][h * DH : (h + 1) * DH, c * 512 : (c + 1) * 512],
                            pO[h][0:DH, :], recb[:], MULT,
                        )

                def outproj_chain(it, dc, stage_eng):
                    def go():
                        po = ps_gen.tile([P, 512], F32, name=f"out{it}_{dc}", tag="gen")
                        for et in range(MT):
                            nc.tensor.matmul(
                                po[:],
                                (OT[et][:, it * P : (it + 1) * P]),
                                (wo[:, et, dc * 512 : (dc + 1) * 512]),
                                start=(et == 0), stop=(et == MT - 1),
                            )
                        osb = opool.tile([P, 512], BF16, name="osb", tag="osb")
                        if stage_eng == "dve":
                            nc.vector.tensor_copy(osb[:], po[:])
                        elif stage_eng == "act":
                            nc.scalar.activation(osb[:], po[:], COPYF)
                        else:
                            nc.gpsimd.tensor_copy(osb[:], po[:])
                        nc.sync.dma_start(
                            out_d[it * P : (it + 1) * P, dc * 512 : (dc + 1) * 512],
                            osb[:],
                        )
                    return go

                def outproj_chains(cp, engs=("dve",)):  # gpsimd cannot read PSUM
                    return [
                        outproj_chain(it, dc, engs[(it + dc) % len(engs)])
                        for it in range(cp * 4, (cp + 1) * 4)
                        for dc in range(DIM // 512)
                    ]

                # chunk order 1,0,2,3: attention(1,hp) starts once only ITS
                # k chunks 0,1 + q chunk 1 are projected (6 chains, k-first);
                # the q(0) chains, qk(2,3) and each finished chunk's out-proj
                # drain as PE fillers inside later attentions' latency bubbles
                def qch(c, mt):
                    return qk_chain(c, wq_sb, qT, cos_s, sin_s, mt)

                def kch(c, mt):
                    return qk_chain(c, wk_sb, kTt, cos_t, sin_t, mt)

                for chain in (kch(0, 0), kch(1, 0), qch(1, 0),
                              kch(0, 1), kch(1, 1), qch(1, 1)):
                    chain()
                filler_q.extend([qch(0, 0), qch(0, 1)])
                filler_q.extend(qk_chains(2) + qk_chains(3))
                for hp in range(MT):
                    attention(1, hp)
                drain_fillers(len(filler_q))  # qk(2,3) complete
                filler_q.extend(outproj_chains(1))
                for hp in range(MT):
                    attention(0, hp)
                filler_q.extend(outproj_chains(0))
                for hp in range(MT):
                    attention(2, hp)
                filler_q.extend(outproj_chains(2))
                for hp in range(MT):
                    attention(3, hp)
                drain_fillers(len(filler_q))
                for chain in outproj_chains(3, ("act", "dve")):
                    chain()


def _rope_tables():
    inv_freq = 1.0 / (ROPE_THETA ** (np.arange(0, DH, 2, dtype=np.float64) / DH))
    t = np.arange(N, dtype=np.float64)
    freqs = t[:, None] * inv_freq[None, :]  # [N, 32]
    cos = np.cos(freqs)
    sin = np.sin(freqs)
    rows = np.arange(P)
    tidx = (rows % DH) // 2
    cos_t = cos[:, tidx].T.astype(np.float32)  # [128, N]
    sign = np.where(rows % 2 == 0, -1.0, 1.0)
    sin_t = (sin[:, tidx] * sign[None, :]).T.astype(np.float32)
    return np.ascontiguousarray(cos_t), np.ascontiguousarray(sin_t)


def shard_inputs(tokens, norm_weight, wq, wk, wv, wo):
    """Build the 8 per-core input dicts (pure numpy layout prep)."""
    import ml_dtypes

    bf16 = ml_dtypes.bfloat16
    tokens = np.asarray(tokens, dtype=np.float32)
    norm_weight = np.asarray(norm_weight, dtype=np.float32)
    wq, wk, wv, wo = (np.asarray(w, dtype=np.float32) for w in (wq, wk, wv, wo))
    # fold norm_weight into the input side of the qkv projections
    wq_f = wq * norm_weight[None, :]
    wk_f = wk * norm_weight[None, :]
    wv_f = wv * norm_weight[None, :]

    cos_t, sin_t = _rope_tables()
    tri = np.triu(np.ones((P, P), dtype=np.float32))  # keep j <= i (row=j, col=i)

    in_maps = []
    for c in range(N_CORES):
        b = c // (N_CORES // B)
        g = c % (N_CORES // B)
        sl = slice(g * M, (g + 1) * M)
        in_maps.append({
            "xT": np.ascontiguousarray(tokens[b].T).astype(bf16),
            "wqT": np.ascontiguousarray(wq_f[sl, :].T).astype(bf16),
            "wkT": np.ascontiguousarray(wk_f[sl, :].T).astype(bf16),
            "wvT": np.ascontiguousarray(wv_f[sl, :].T).astype(bf16),
            "woT": np.ascontiguousarray(wo[:, sl].T).astype(bf16),
            "cos_t": cos_t.astype(bf16),
            "sin_t": sin_t.astype(bf16),
            "tri": tri.astype(bf16),
        })
    return in_maps


_PROGRAM = None


def _get_program():
    global _PROGRAM
    if _PROGRAM is None:
        _PROGRAM = build_program()
    return _PROGRAM


def run(tokens, norm_weight, wq, wk, wv, wo, trace=False, **run_kwargs):
    nc = _get_program()
    in_maps = shard_inputs(tokens, norm_weight, wq, wk, wv, wo)
    res = run_bass_kernel_spmd(
        nc, in_maps, core_ids=list(range(N_CORES)), trace=trace, **run_kwargs
    )
    parts = [r["out_part"] for r in res.results]
    out = np.zeros((B, N, DIM), dtype=np.float64)
    for c in range(N_CORES):
        out[c // (N_CORES // B)] += parts[c].astype(np.float64)
    return out.astype(np.float32), res


def kernel(tokens, norm_weight, wq, wk, wv, wo):
    out, _ = run(tokens, norm_weight, wq, wk, wv, wo)
    return out


if __name__ == "__main__":
    from concourse.timeline_sim import TimelineSim

    nc = build_program()
    ts = TimelineSim(nc, trace=False)
    print(f"TimelineSim: {ts.simulate():.0f} ns")
